# revision 18
# baseline (speedup 1.0000x reference)
"""Trainium2 Bass kernel for nn_IdentityConvolution.

reference semantics:
    r = sum_c x_real[b, c, :, :]   # [B, 1, H, W]
    i = sum_c x_imag[b, c, :, :]
    out = complex(r, i) broadcast to [B, 64, H, W]  (complex64)

Sharding: data-parallel over batch B=8 across the 8 NeuronCores (one
batch image per core, no cross-core communication).

Per-core device program (Tile-scheduled), built around a small number of
large, deeply-pipelined operations so no engine accumulates short
serialization slices:

  - inputs viewed as [C=64, P=128, Q=512] (hw = p*512 + q), processed in
    nred q-blocks of qb per lane (real/imag).
  - per block+lane: one DMA loads [128 p, 64 c, qb] f32 (1KB contiguous
    runs), then a single DVE tensor_reduce over the channel axis (via a
    "p c q -> p q c" strided view, axis=X) writes the 128 x qb channel
    sums straight into the complex-interleaved SBUF output tile
    (strided out AP, t=0 real / t=1 imag). fp32 accumulation keeps the
    sums fp32-exact.
  - a narrow Pool tensor_copy snapshots a strip of the first result tile
    (cheap result probe; also the only short engine op in the program).
  - each [128, 2*qb] f32 output tile is broadcast-DMA'd to all 64 output
    channel planes (stride-0 source AP) on the SP/Act queues.
"""

import sys

sys.path.insert(0, "/opt/trn_rl_repo")

from contextlib import ExitStack

import numpy as np

import concourse.bacc as bacc
import concourse.tile as tile
from concourse import mybir
from concourse.bass_utils import run_bass_kernel_spmd

B, C, H, W = 8, 64, 256, 256
P = 128
Q = (H * W) // P  # 512
NRED = 2  # q blocks per lane
QB = Q // NRED  # 256

F32 = mybir.dt.float32

_cache = {}


def _build_program(
    repeat=1,
    barrier=False,
    nred=NRED,
    out_bcast=32,  # output channel planes per broadcast DMA
    inbufs=2,
    probe_cols=8,  # width of the Pool result-probe copy
):
    qb = Q // nred
    nc = bacc.Bacc("TRN2", target_bir_lowering=False, debug=False, num_devices=8)
    xr = nc.dram_tensor("x_real", [C, P, Q], F32, kind="ExternalInput").ap()
    xi = nc.dram_tensor("x_imag", [C, P, Q], F32, kind="ExternalInput").ap()
    out = nc.dram_tensor("out", [C, P, 2 * Q], F32, kind="ExternalOutput").ap()

    with tile.TileContext(nc) as tc, ExitStack() as ctx:
        inp = ctx.enter_context(tc.tile_pool(name="inp", bufs=inbufs))
        outp = ctx.enter_context(tc.tile_pool(name="outp", bufs=2))
        stgp = ctx.enter_context(tc.tile_pool(name="stg", bufs=1))

        for r in range(repeat):
            if r and barrier:
                tc.strict_bb_all_engine_barrier()
            for o in range(nred):
                q0 = o * qb
                ot = outp.tile([P, 2 * qb], F32, tag="ot")
                otv = ot[:].rearrange("p (q t) -> p q t", t=2)
                for t, x in enumerate((xr, xi)):
                    xt = inp.tile([P, C, qb], F32, tag="in")
                    (nc.sync, nc.scalar)[t].dma_start(
                        out=xt[:],
                        in_=x[:, :, q0 : q0 + qb].rearrange("c p q -> p c q"),
                    )
                    nc.vector.tensor_reduce(
                        out=otv[:, :, t],
                        in_=xt[:].rearrange("p c q -> p q c"),
                        axis=mybir.AxisListType.X,
                        op=mybir.AluOpType.add,
                    )
                if r == 0 and o == 0 and probe_cols:
                    # snapshot a strip of the first result tile (Pool)
                    stg = stgp.tile([P, probe_cols], F32, tag="stg")
                    nc.gpsimd.tensor_copy(out=stg[:], in_=ot[:, :probe_cols])
                for m, co in enumerate(range(0, C, out_bcast)):
                    (nc.sync, nc.scalar)[m % 2].dma_start(
                        out=out[co : co + out_bcast, :, 2 * q0 : 2 * q0 + 2 * qb]
                        .rearrange("c p q -> p c q"),
                        in_=ot[:].unsqueeze(1).broadcast_to((P, out_bcast, 2 * qb)),
                    )
    nc.compile()
    return nc


def kernel(x_real, x_imag, _profile=False):
    if "nc" not in _cache:
        _cache["nc"] = _build_program()
    nc = _cache["nc"]

    x_real = np.asarray(x_real)
    x_imag = np.asarray(x_imag)
    in_maps = [
        {
            "x_real": np.ascontiguousarray(x_real[b]).reshape(C, P, Q),
            "x_imag": np.ascontiguousarray(x_imag[b]).reshape(C, P, Q),
        }
        for b in range(B)
    ]
    res = run_bass_kernel_spmd(nc, in_maps, list(range(B)), trace=_profile)
    _cache["last_result"] = res

    out = np.empty((B, C, H, W), dtype=np.complex64)
    for b in range(B):
        o = res.results[b]["out"]  # [C, P, 2Q] f32
        out[b] = o.reshape(C, P * Q, 2).view(np.complex64).reshape(C, H, W)
    return out
